# revision 9
# baseline (speedup 1.0000x reference)
"""DAS kernel v8: GPSIMD ap_gather-based.

Per core (8192 pixels, all 128 detectors):
- sino is host-relayouted (lut-independent) into windowed tables
  sgw[set, ch, t, 0:2] = (S[b, det, t], S[b, det, t+1]) in bf16, where
  set i covers detectors 8i..8i+7 (one per 16-partition group g) and
  channel ch = 16g + 4r + b holds batch b (replicated r=0..3).
- ap_gather (d=2) pulls both lerp samples for 4096 pixels x 8 detectors
  per instruction using per-detector k0 index streams (static, from lut).
- DVE lerp with host-baked alpha, then accumulating PE matmuls
  (apod one-hot lhsT) reduce over detectors into PSUM across the 16 sets.
"""
import numpy as np

import concourse.bass as bass
import concourse.tile as tile
from concourse import bacc, mybir

N_DET, N_T, NY, NX, B = 128, 2048, 256, 256, 4
P_TOTAL = NY * NX
N_CORES = 8
PX_PER_CORE = P_TOTAL // N_CORES
N_SETS = 16          # detector sets of 8 (one det per 16-partition group)
N_HALF = 2           # pixel halves per core
PX_HALF = PX_PER_CORE // N_HALF  # 4096 pixels per gather instruction
F32 = mybir.dt.float32
BF16 = mybir.dt.bfloat16
I16 = mybir.dt.int16


def _build_kernel():
    nc = bacc.Bacc("TRN2", target_bir_lowering=False, debug=False)

    sgw = nc.dram_tensor("sgw", [N_SETS, 128, N_T * 2], BF16,
                         kind="ExternalInput")
    idxs = nc.dram_tensor("idxs", [N_HALF, N_SETS, 128, PX_HALF // 16], I16,
                          kind="ExternalInput")
    alph = nc.dram_tensor("alph", [N_HALF, N_SETS, 128, PX_HALF], BF16,
                          kind="ExternalInput")
    apodT = nc.dram_tensor("apodT", [128, N_SETS * 4], BF16,
                           kind="ExternalInput")
    outd = nc.dram_tensor("out", [N_HALF, 4, PX_HALF], F32,
                          kind="ExternalOutput")

    n_q = PX_HALF // 512

    with tile.TileContext(nc) as tc:
        with (
            tc.tile_pool(name="const", bufs=1) as cpool,
            tc.tile_pool(name="tabs", bufs=2) as tabs,
            tc.tile_pool(name="io", bufs=2) as io,
            tc.tile_pool(name="gat", bufs=2) as gat,
            tc.tile_pool(name="de", bufs=2) as de,
            tc.tile_pool(name="ps", bufs=1, space="PSUM") as ps,
            tc.tile_pool(name="oc", bufs=2) as oc,
        ):
            apod_tl = cpool.tile([128, N_SETS * 4], BF16)
            nc.sync.dma_start(out=apod_tl[:], in_=apodT.ap())

            for h in range(N_HALF):
                psq = []
                for q in range(n_q):
                    pst = ps.tile([4, 512], F32, tag=f"ps{q}", name=f"ps{q}")
                    psq.append(pst)
                for i in range(N_SETS):
                    tab = tabs.tile([128, N_T * 2], BF16, tag="tab")
                    nc.sync.dma_start(out=tab[:], in_=sgw.ap()[i])
                    idx_t = io.tile([128, PX_HALF // 16], I16, tag="idx")
                    nc.sync.dma_start(out=idx_t[:], in_=idxs.ap()[h, i])
                    al_t = io.tile([128, PX_HALF], BF16, tag="al")
                    nc.sync.dma_start(out=al_t[:], in_=alph.ap()[h, i])

                    G = gat.tile([128, PX_HALF * 2], BF16, tag="G")
                    nc.gpsimd.ap_gather(
                        out_ap=G[:], in_ap=tab[:], idxs_ap=idx_t[:],
                        channels=128, num_elems=N_T, d=2, num_idxs=PX_HALF)

                    g3 = G[:].rearrange("c (p w) -> c p w", w=2)
                    g0 = g3[:, :, 0:1]
                    g1 = g3[:, :, 1:2]
                    a3 = bass.AP(al_t.tensor, al_t[:].offset,
                                 [al_t[:].ap[0], [1, PX_HALF], [0, 1]])
                    # F = g0 + alpha*(g1-g0)
                    D = de.tile([128, PX_HALF], BF16, tag="D")
                    d3 = D[:].rearrange("c (p w) -> c p w", w=1)
                    nc.vector.tensor_tensor(out=d3, in0=g1, in1=g0,
                                            op=mybir.AluOpType.subtract)
                    E = de.tile([128, PX_HALF], BF16, tag="E")
                    e3 = E[:].rearrange("c (p w) -> c p w", w=1)
                    nc.vector.tensor_tensor(out=e3, in0=d3, in1=a3,
                                            op=mybir.AluOpType.mult)
                    F = de.tile([128, PX_HALF], BF16, tag="F")
                    f3 = F[:].rearrange("c (p w) -> c p w", w=1)
                    nc.vector.tensor_tensor(out=f3, in0=e3, in1=g0,
                                            op=mybir.AluOpType.add)

                    for q in range(n_q):
                        nc.tensor.matmul(
                            out=psq[q][:], lhsT=apod_tl[:, i * 4:(i + 1) * 4],
                            rhs=F[:, q * 512:(q + 1) * 512],
                            start=(i == 0), stop=(i == N_SETS - 1))

                outc = oc.tile([4, PX_HALF], F32, tag="outc")
                for q in range(n_q):
                    nc.scalar.copy(out=outc[:, q * 512:(q + 1) * 512],
                                   in_=psq[q][:])
                nc.sync.dma_start(out=outd.ap()[h], in_=outc[:])

    nc.compile()
    return nc


def _host_prep(sino: np.ndarray, lut: np.ndarray):
    import ml_dtypes
    bf16 = ml_dtypes.bfloat16

    sino = np.ascontiguousarray(sino, dtype=np.float32)
    lut = np.ascontiguousarray(lut, dtype=np.float32)
    S = sino[:, 0]  # [B, N_DET, N_T]

    # windowed, replicated sino tables (lut-independent relayout)
    # sgw[i, 16g+4r+b, t, w] = S[b, 8i+g, t+w] (zero-padded at t+1==N_T)
    Spad = np.zeros((B, N_DET, N_T + 1), dtype=np.float32)
    Spad[:, :, :N_T] = S
    win = np.stack([Spad[:, :, :N_T], Spad[:, :, 1:N_T + 1]], axis=-1)
    # [B, det, t, 2] -> [set, g, r, b, t, 2]
    win = win.reshape(B, N_SETS, 8, N_T, 2)
    sgw = np.broadcast_to(win[:, None, :, :, :, :].transpose(2, 3, 1, 0, 4, 5),
                          (N_SETS, 8, 4, B, N_T, 2))
    sgw = np.ascontiguousarray(sgw.reshape(N_SETS, 128, N_T * 2)).astype(bf16)

    lut_flat = lut.reshape(P_TOTAL, N_DET, 2)
    k_floor = np.floor(lut_flat[:, :, 0])
    valid = (k_floor >= 0) & (k_floor < N_T - 1)
    k0 = np.clip(k_floor, 0, N_T - 2).astype(np.int64)   # [P, det]
    alpha = np.where(valid, lut_flat[:, :, 1], 0.0)      # [P, det]

    apod = (0.5 - 0.5 * np.cos(
        2.0 * np.pi * np.arange(N_DET, dtype=np.float32) / (N_DET - 1)
    )).astype(np.float32)
    norm = max(apod.sum(), np.finfo(np.float32).tiny)
    apod_n = apod / norm  # [det]

    # apodT[16g+c, i*4+b] = apod_n[8i+g] if c == b else 0
    apodT = np.zeros((N_SETS, 8, 16, 4), dtype=np.float32)
    for b in range(4):
        apodT[:, :, b, b] = apod_n.reshape(N_SETS, 8)
    apodT = np.ascontiguousarray(
        apodT.transpose(1, 2, 0, 3).reshape(128, N_SETS * 4)).astype(bf16)

    in_maps = []
    for c in range(N_CORES):
        sl = slice(c * PX_PER_CORE, (c + 1) * PX_PER_CORE)
        k0c = k0[sl]          # [8192, det]
        alc = alpha[sl]       # [8192, det]
        # idx[h, i, 16g+p, s] = k0[h*4096 + 16s + p, 8i+g]
        k0h = k0c.reshape(N_HALF, PX_HALF // 16, 16, N_SETS, 8)
        idx = np.ascontiguousarray(
            k0h.transpose(0, 3, 4, 2, 1)).astype(np.int16)
        idx = idx.reshape(N_HALF, N_SETS, 128, PX_HALF // 16)
        # alph[h, i, ch, j] = alpha[h*4096 + j, 8i+g] for every ch in group g
        alh = alc.reshape(N_HALF, PX_HALF, N_SETS, 8)
        al = np.broadcast_to(
            alh.transpose(0, 2, 3, 1)[:, :, :, None, :],
            (N_HALF, N_SETS, 8, 16, PX_HALF))
        al = np.ascontiguousarray(
            al.reshape(N_HALF, N_SETS, 128, PX_HALF)).astype(bf16)
        in_maps.append({
            "sgw": sgw,
            "idxs": idx,
            "alph": al,
            "apodT": apodT,
        })
    return in_maps


def _assemble(results: list) -> np.ndarray:
    # per core out [2, 4, 4096] -> pixels c*8192 + h*4096 + j, batch b
    full = np.empty((B, P_TOTAL), dtype=np.float32)
    for c, r in enumerate(results):
        o = r["out"]  # [2, 4, 4096]
        for h in range(N_HALF):
            base = c * PX_PER_CORE + h * PX_HALF
            full[:, base:base + PX_HALF] = o[h]
    return np.ascontiguousarray(full).reshape(B, 1, NY, NX)


_CACHE: dict = {}


def _get_nc():
    if "nc" not in _CACHE:
        _CACHE["nc"] = _build_kernel()
    return _CACHE["nc"]


def kernel(sino: np.ndarray, lut: np.ndarray) -> np.ndarray:
    from concourse.bass_utils import run_bass_kernel_spmd

    nc = _get_nc()
    in_maps = _host_prep(np.asarray(sino), np.asarray(lut))
    res = run_bass_kernel_spmd(nc, in_maps, core_ids=list(range(N_CORES)))
    return _assemble(res.results)


def kernel_timed(inputs: dict, iters: int = 20) -> float:
    """Run the kernel repeatedly with device-resident inputs; return ns/iter."""
    import time
    import jax
    from jax.sharding import Mesh, PartitionSpec
    from jax.experimental.shard_map import shard_map
    from concourse.bass2jax import (
        _bass_exec_p, install_neuronx_cc_hook)
    import concourse.mybir as mybir_

    nc = _get_nc()
    in_maps = _host_prep(np.asarray(inputs["sino"]), np.asarray(inputs["lut"]))

    install_neuronx_cc_hook()
    part_name = nc.partition_id_tensor.name if nc.partition_id_tensor else None
    in_names, out_names, out_avals, zero_outs = [], [], [], []
    for alloc in nc.m.functions[0].allocations:
        if not isinstance(alloc, mybir_.MemoryLocationSet):
            continue
        name = alloc.memorylocations[0].name
        if alloc.kind == "ExternalInput":
            if name != part_name:
                in_names.append(name)
        elif alloc.kind == "ExternalOutput":
            out_names.append(name)
            shape = tuple(alloc.tensor_shape)
            dtype = mybir_.dt.np(alloc.dtype)
            out_avals.append(jax.core.ShapedArray(shape, dtype))
            zero_outs.append(np.zeros(shape, dtype))
    n_params = len(in_names)
    all_names = in_names + out_names
    if part_name is not None:
        all_names.append(part_name)
    from concourse.bass2jax import partition_id_tensor

    def _body(*args):
        operands = list(args)
        if part_name is not None:
            operands.append(partition_id_tensor())
        outs = _bass_exec_p.bind(
            *operands,
            out_avals=tuple(out_avals),
            in_names=tuple(all_names),
            out_names=tuple(out_names),
            lowering_input_output_aliases=(),
            sim_require_finite=True,
            sim_require_nnan=True,
            nc=nc,
        )
        return tuple(outs)

    devices = jax.devices()[:N_CORES]
    mesh = Mesh(np.asarray(devices), ("core",))
    n_outs = len(out_names)
    sharded = jax.jit(
        shard_map(_body, mesh=mesh,
                  in_specs=(PartitionSpec("core"),) * (n_params + n_outs),
                  out_specs=(PartitionSpec("core"),) * n_outs,
                  check_rep=False),
        keep_unused=True,
    )
    concat_in = [
        np.concatenate([in_maps[c][name] for c in range(N_CORES)], axis=0)
        for name in in_names
    ]
    concat_zeros = [
        np.zeros((N_CORES * z.shape[0], *z.shape[1:]), z.dtype) for z in zero_outs
    ]
    dev_in = [jax.device_put(a) for a in concat_in]
    dev_zero = [jax.device_put(a) for a in concat_zeros]

    # warmup (compile + 2 runs)
    for _ in range(3):
        outs = sharded(*dev_in, *dev_zero)
        jax.block_until_ready(outs)

    t0 = time.perf_counter()
    for _ in range(iters):
        outs = sharded(*dev_in, *dev_zero)
    jax.block_until_ready(outs)
    t1 = time.perf_counter()
    return (t1 - t0) / iters * 1e9


# revision 33
# speedup vs baseline: 2.2934x; 2.2934x over previous
"""DAS kernel v9: ap_gather + minimal HBM traffic.

The 8-core-concurrent DRAM pipe is ~11 GB/s aggregate, so per-iteration
bytes dominate. Per core we stream only:
  sino_bf [128, 8192] bf16 (2 MB)  sino_bf[det, 4t+b] = S[b, det, t]
  idxs    [2,16,128,256] i16 (2 MB) per-detector k0 streams (16-wrapped)
  alpha8  [2,16,8,4096] u8  (1 MB)  alpha quantized to u8
Windowed + replicated gather tables (ap_gather layout: channel 16g+4r+b,
free (t, w)) and the 16x alpha replication are built on-chip via
SBUF->SBUF DMAs; alpha is dequantized on DVE.  Gather: GPSIMD ap_gather
(d=2) -> DVE lerp -> accumulating PE matmuls over 16 detector sets.
"""
import os
import numpy as np

import concourse.bass as bass
import concourse.tile as tile
from concourse import bacc, mybir

N_DET, N_T, NY, NX, B = 128, 2048, 256, 256, 4
P_TOTAL = NY * NX
N_CORES = 8
PX_PER_CORE = P_TOTAL // N_CORES
N_SETS = 16          # detector sets of 8 (one det per 16-partition group)
N_HALF = 2           # pixel halves per core
PX_HALF = PX_PER_CORE // N_HALF  # 4096 pixels per gather instruction
F32 = mybir.dt.float32
BF16 = mybir.dt.bfloat16
I16 = mybir.dt.int16
U8 = mybir.dt.uint8


def _ap(t, offset, dims):
    return bass.AP(t.tensor if hasattr(t, "tensor") else t, offset, dims)


def _build_kernel():
    nc = bacc.Bacc("TRN2", target_bir_lowering=False, debug=False,
                   detect_race_conditions=False)

    TP = N_T + 8  # per-b padded stride in sino_bf
    sino_bf = nc.dram_tensor("sino_bf", [128, B * TP], BF16,
                             kind="ExternalInput")
    idxs = nc.dram_tensor("idxs", [N_HALF, N_SETS, 128, PX_HALF // 16], I16,
                          kind="ExternalInput")
    alpha8 = nc.dram_tensor("alpha8", [N_HALF, N_SETS, 8, PX_HALF], U8,
                            kind="ExternalInput")
    apodT = nc.dram_tensor("apodT", [128, N_SETS * 4], BF16,
                           kind="ExternalInput")
    place = nc.dram_tensor("place", [8, B * 128], BF16, kind="ExternalInput")
    rep8 = nc.dram_tensor("rep8", [8, 128], BF16, kind="ExternalInput")
    outd = nc.dram_tensor("out", [N_HALF, 4, PX_HALF], F32,
                          kind="ExternalOutput")

    n_q = PX_HALF // 512
    NT1 = N_T + 1
    n_tc = (NT1 + 511) // 512  # tabP column chunks (4x512 + 1)

    with tile.TileContext(nc) as tc:
        sino_t = nc.alloc_sbuf_tensor("sino_t", [128, B * TP], BF16)
        apod_tl = nc.alloc_sbuf_tensor("apod_tl", [128, N_SETS * 4], BF16)
        place_t = nc.alloc_sbuf_tensor("place_t", [8, B * 128], BF16)
        rep8_t = nc.alloc_sbuf_tensor("rep8_t", [8, 128], BF16)
        tabP = [nc.alloc_sbuf_tensor(f"tabP{p}", [128, TP], BF16)
                for p in range(2)]
        tab = [nc.alloc_sbuf_tensor(f"tab{p}", [128, N_T * 2], BF16)
               for p in range(2)]
        idx_t = [nc.alloc_sbuf_tensor(f"idx{p}", [128, PX_HALF // 16], I16)
                 for p in range(2)]
        al8s = [nc.alloc_sbuf_tensor(f"al8s{p}", [8, PX_HALF], U8)
                for p in range(2)]
        albf8 = [nc.alloc_sbuf_tensor(f"albf8{p}", [8, PX_HALF], BF16)
                 for p in range(2)]
        Gt = [nc.alloc_sbuf_tensor(f"G{p}", [128, PX_HALF * 2], BF16)
              for p in range(2)]
        Dt = nc.alloc_sbuf_tensor("Dt", [128, PX_HALF], BF16)
        Et = nc.alloc_sbuf_tensor("Et", [128, PX_HALF], BF16)
        Ft = [nc.alloc_sbuf_tensor(f"F{p}", [128, PX_HALF], BF16)
              for p in range(2)]
        acc = [nc.alloc_sbuf_tensor(f"acc{p}", [4, PX_HALF], F32)
               for p in range(2)]
        stg = [nc.alloc_sbuf_tensor(f"stg{p}", [8, B * TP], BF16)
               for p in range(2)]

        with tc.tile_pool(name="ps", bufs=1, space="PSUM") as ps:
            nc.sync.dma_start(out=apod_tl[:], in_=apodT.ap())
            nc.sync.dma_start(out=place_t[:], in_=place.ap())
            nc.sync.dma_start(out=rep8_t[:], in_=rep8.ap())
            nc.sync.dma_start(out=sino_t[:], in_=sino_bf.ap())

            # psum scratch: 2 full-partition tiles for table build, 2 for
            # alpha, 2 [4,512] for the apod reduction
            tps = []
            for p in range(2):
                t_ = ps.tile([128, 512], F32, tag=f"tb{p}", name=f"tb{p}")
                tps.append(t_)
            aps_ = []
            for p in range(2):
                t_ = ps.tile([128, 512], F32, tag=f"al{p}", name=f"al{p}")
                aps_.append(t_)
            ops_ = []
            for p in range(2):
                t_ = ps.tile([4, 512], F32, tag=f"oq{p}", name=f"oq{p}")
                ops_.append(t_)

            for h in range(N_HALF):
                ac = acc[h % 2]
                for i in range(N_SETS):
                    p = i % 2
                    tP, tb, ix, a8, ab8, G, F, sg = (
                        tabP[p], tab[p], idx_t[p], al8s[p], albf8[p],
                        Gt[p], Ft[p], stg[p])
                    # --- tabP via accumulated placement matmuls ---
                    # tabP[16g+4b+r, t] = S[b, 8i+g, t]
                    nc.scalar.dma_start(out=sg[:],
                                        in_=sino_t[8 * i:8 * i + 8])
                    for q in range(n_tc):
                        cs = q * 512
                        ncol = min(512, NT1 - cs)
                        pt = tps[q % 2]
                        for b in range(B):
                            nc.tensor.matmul(
                                out=pt[:, :ncol],
                                lhsT=place_t[:, b * 128:(b + 1) * 128],
                                rhs=sg[0:8,
                                       b * TP + cs:b * TP + cs + ncol],
                                start=(b == 0), stop=(b == B - 1))
                        nc.scalar.copy(out=tP[:, cs:cs + ncol],
                                       in_=pt[:, :ncol])
                    # window interleave on DVE: tab[c, 2t+w] = tabP[c, t+w]
                    row_t = tb[:].ap[0][0]
                    rowP = tP[:].ap[0][0]
                    for w in range(2):
                        dstw = bass.AP(tb, w, [[row_t, 128], [2, N_T]])
                        srcw = bass.AP(tP, w, [[rowP, 128], [1, N_T]])
                        nc.vector.tensor_copy(out=dstw, in_=srcw)

                    nc.sync.dma_start(out=ix[:], in_=idxs.ap()[h, i])
                    nc.sync.dma_start(out=a8[:], in_=alpha8.ap()[h, i])
                    nc.vector.tensor_scalar(
                        out=ab8[:], in0=a8[:], scalar1=1.0 / 255.0,
                        scalar2=0.0, op0=mybir.AluOpType.mult,
                        op1=mybir.AluOpType.add)

                    nc.gpsimd.ap_gather(
                        out_ap=G[:], in_ap=tb[:], idxs_ap=ix[:],
                        channels=128, num_elems=N_T, d=2, num_idxs=PX_HALF)

                    g3 = G[:].rearrange("c (p w) -> c p w", w=2)
                    g0 = g3[:, :, 0:1]
                    g1 = g3[:, :, 1:2]
                    # F = g0 + alpha*(g1-g0); alpha replicated via matmul,
                    # consumed straight from PSUM per 512-px chunk
                    d3 = Dt[:].rearrange("c (p w) -> c p w", w=1)
                    nc.vector.tensor_tensor(out=d3, in0=g1, in1=g0,
                                            op=mybir.AluOpType.subtract)
                    for q in range(n_q):
                        cs = q * 512
                        at = aps_[q % 2]
                        nc.tensor.matmul(
                            out=at[:], lhsT=rep8_t[:],
                            rhs=ab8[0:8, cs:cs + 512],
                            start=True, stop=True)
                        nc.vector.tensor_tensor(
                            out=Et[:, cs:cs + 512], in0=Dt[:, cs:cs + 512],
                            in1=at[:], op=mybir.AluOpType.mult)
                    f3 = F[:].rearrange("c (p w) -> c p w", w=1)
                    e3 = Et[:].rearrange("c (p w) -> c p w", w=1)
                    nc.vector.tensor_tensor(out=f3, in0=e3, in1=g0,
                                            op=mybir.AluOpType.add)

                    for q in range(n_q):
                        cs = q * 512
                        ot = ops_[q % 2]
                        nc.tensor.matmul(
                            out=ot[:], lhsT=apod_tl[:, i * 4:(i + 1) * 4],
                            rhs=F[:, cs:cs + 512], start=True, stop=True)
                        if i == 0:
                            nc.vector.tensor_copy(out=ac[:, cs:cs + 512],
                                                  in_=ot[:])
                        else:
                            nc.vector.tensor_tensor(
                                out=ac[:, cs:cs + 512],
                                in0=ac[:, cs:cs + 512], in1=ot[:],
                                op=mybir.AluOpType.add)

                nc.sync.dma_start(out=outd.ap()[h], in_=ac[:])

    nc.compile()
    return nc


def _host_prep(sino: np.ndarray, lut: np.ndarray):
    import ml_dtypes
    bf16 = ml_dtypes.bfloat16

    sino = np.ascontiguousarray(sino, dtype=np.float32)
    lut = np.ascontiguousarray(lut, dtype=np.float32)
    S = sino[:, 0]  # [B, N_DET, N_T]

    # sino_bf[det, b*(N_T+8)+t] = S[b, det, t], zero padded
    TP = N_T + 8
    sino_pad = np.zeros((128, B, TP), dtype=np.float32)
    sino_pad[:, :, :N_T] = S.transpose(1, 0, 2)
    sino_bf = np.ascontiguousarray(sino_pad.reshape(128, B * TP)).astype(bf16)

    lut_flat = lut.reshape(P_TOTAL, N_DET, 2)
    k_floor = np.floor(lut_flat[:, :, 0])
    valid = (k_floor >= 0) & (k_floor < N_T - 1)
    k0 = np.clip(k_floor, 0, N_T - 2).astype(np.int64)   # [P, det]
    alpha = np.where(valid, lut_flat[:, :, 1], 0.0)      # [P, det]
    alpha_q = np.round(alpha * 255.0).astype(np.uint8)

    apod = (0.5 - 0.5 * np.cos(
        2.0 * np.pi * np.arange(N_DET, dtype=np.float32) / (N_DET - 1)
    )).astype(np.float32)
    norm = max(apod.sum(), np.finfo(np.float32).tiny)
    apod_n = apod / norm  # [det]

    # apodT[16g+c, i*4+b] = apod_n[8i+g] if c == b else 0
    apodT = np.zeros((N_SETS, 8, 16, 4), dtype=np.float32)
    for b in range(4):
        apodT[:, :, 4 * b, b] = apod_n.reshape(N_SETS, 8)
    apodT = np.ascontiguousarray(
        apodT.transpose(1, 2, 0, 3).reshape(128, N_SETS * 4)).astype(bf16)

    # placement one-hots: place[g, b*128 + (16g+4b+r)] = 1
    place = np.zeros((8, B, 16, 8), dtype=np.float32)  # [g, b, c%16? ...]
    place = np.zeros((8, B * 128), dtype=np.float32)
    for g in range(8):
        for b in range(B):
            for r in range(4):
                place[g, b * 128 + 16 * g + 4 * b + r] = 1.0
    place = place.astype(bf16)
    # rep8[g, c] = 1 if c//16 == g
    rep8 = np.zeros((8, 128), dtype=np.float32)
    for g in range(8):
        rep8[g, 16 * g:16 * g + 16] = 1.0
    rep8 = rep8.astype(bf16)

    in_maps = []
    for c in range(N_CORES):
        sl = slice(c * PX_PER_CORE, (c + 1) * PX_PER_CORE)
        k0c = k0[sl]          # [8192, det]
        alc = alpha_q[sl]     # [8192, det] u8
        # idx[h, i, 16g+p, s] = k0[h*4096 + 16s + p, 8i+g]
        k0h = k0c.reshape(N_HALF, PX_HALF // 16, 16, N_SETS, 8)
        idx = np.ascontiguousarray(
            k0h.transpose(0, 3, 4, 2, 1)).astype(np.int16)
        idx = idx.reshape(N_HALF, N_SETS, 128, PX_HALF // 16)
        # alpha8[h, i, g, j] = alpha_q[h*4096 + j, 8i+g]
        alh = alc.reshape(N_HALF, PX_HALF, N_SETS, 8)
        al = np.ascontiguousarray(alh.transpose(0, 2, 3, 1))
        in_maps.append({
            "sino_bf": sino_bf,
            "idxs": idx,
            "alpha8": al,
            "apodT": apodT,
            "place": place,
            "rep8": rep8,
        })
    return in_maps


def _assemble(results: list) -> np.ndarray:
    full = np.empty((B, P_TOTAL), dtype=np.float32)
    for c, r in enumerate(results):
        o = r["out"]  # [2, 4, 4096]
        for h in range(N_HALF):
            base = c * PX_PER_CORE + h * PX_HALF
            full[:, base:base + PX_HALF] = o[h]
    return np.ascontiguousarray(full).reshape(B, 1, NY, NX)


_CACHE: dict = {}


def _get_nc():
    if "nc" not in _CACHE:
        _CACHE["nc"] = _build_kernel()
    return _CACHE["nc"]


def kernel(sino: np.ndarray, lut: np.ndarray) -> np.ndarray:
    from concourse.bass_utils import run_bass_kernel_spmd

    nc = _get_nc()
    in_maps = _host_prep(np.asarray(sino), np.asarray(lut))
    res = run_bass_kernel_spmd(nc, in_maps, core_ids=list(range(N_CORES)))
    return _assemble(res.results)


def kernel_timed(inputs: dict, iters: int = 20) -> float:
    """Run the kernel repeatedly with device-resident inputs; return ns/iter."""
    import time
    import jax
    from jax.sharding import Mesh, PartitionSpec
    from jax.experimental.shard_map import shard_map
    from concourse.bass2jax import (
        _bass_exec_p, install_neuronx_cc_hook)
    import concourse.mybir as mybir_

    nc = _get_nc()
    in_maps = _host_prep(np.asarray(inputs["sino"]), np.asarray(inputs["lut"]))

    install_neuronx_cc_hook()
    part_name = nc.partition_id_tensor.name if nc.partition_id_tensor else None
    in_names, out_names, out_avals, zero_outs = [], [], [], []
    for alloc in nc.m.functions[0].allocations:
        if not isinstance(alloc, mybir_.MemoryLocationSet):
            continue
        name = alloc.memorylocations[0].name
        if alloc.kind == "ExternalInput":
            if name != part_name:
                in_names.append(name)
        elif alloc.kind == "ExternalOutput":
            out_names.append(name)
            shape = tuple(alloc.tensor_shape)
            dtype = mybir_.dt.np(alloc.dtype)
            out_avals.append(jax.core.ShapedArray(shape, dtype))
            zero_outs.append(np.zeros(shape, dtype))
    n_params = len(in_names)
    all_names = in_names + out_names
    if part_name is not None:
        all_names.append(part_name)
    from concourse.bass2jax import partition_id_tensor

    def _body(*args):
        operands = list(args)
        if part_name is not None:
            operands.append(partition_id_tensor())
        outs = _bass_exec_p.bind(
            *operands,
            out_avals=tuple(out_avals),
            in_names=tuple(all_names),
            out_names=tuple(out_names),
            lowering_input_output_aliases=(),
            sim_require_finite=True,
            sim_require_nnan=True,
            nc=nc,
        )
        return tuple(outs)

    devices = jax.devices()[:N_CORES]
    mesh = Mesh(np.asarray(devices), ("core",))
    n_outs = len(out_names)
    sharded = jax.jit(
        shard_map(_body, mesh=mesh,
                  in_specs=(PartitionSpec("core"),) * (n_params + n_outs),
                  out_specs=(PartitionSpec("core"),) * n_outs,
                  check_rep=False),
        keep_unused=True,
    )
    concat_in = [
        np.concatenate([in_maps[c][name] for c in range(N_CORES)], axis=0)
        for name in in_names
    ]
    concat_zeros = [
        np.zeros((N_CORES * z.shape[0], *z.shape[1:]), z.dtype) for z in zero_outs
    ]
    dev_in = [jax.device_put(a) for a in concat_in]
    dev_zero = [jax.device_put(a) for a in concat_zeros]

    # warmup (compile + 2 runs)
    for _ in range(3):
        outs = sharded(*dev_in, *dev_zero)
        jax.block_until_ready(outs)

    t0 = time.perf_counter()
    for _ in range(iters):
        outs = sharded(*dev_in, *dev_zero)
    jax.block_until_ready(outs)
    t1 = time.perf_counter()
    return (t1 - t0) / iters * 1e9
